# revision 36
# baseline (speedup 1.0000x reference)
"""BertBlock kernel for 8 Trainium2 NeuronCores.

Sharding: pure data-parallel over (batch, half-sequence) tokens: core c
handles batch element c//2, query-token half c%2 (1024 tokens). Each core
recomputes K/V for the full 2048-token sequence of its batch element, so
no collectives are needed.

Device layout is feature-major ([feature, token]) end to end. The softmax
exp is the Act-engine bottleneck, so scores are staged: Pool/Vector copy
the PSUM score tiles to SBUF bf16 and the Act engine runs one wide exp per
half-head ([128, 8*1024], in place), amortizing the per-instruction
overhead 8x. Softmax denominators come from ones-columns in the V
stationary operand; the V layout per key-chunk pair is
[A.dims | A.ones | B.ones | B.dims] so the odd head's stationary window
(shifted by 2) lands its output rows at partitions 64:128 directly - no
partition-shift DMAs. Reciprocals use the fast approximate DVE op, and
the reciprocal row is partition-broadcast with a ones-column PE matmul.
AV for head h-1 is emitted after scores of head h so the exp latency
never stalls the PE.
"""

import numpy as np
import ml_dtypes

P = 128
B = 4
S = 2048          # sequence length (keys)
SQ = 1024         # query tokens per core
H = 768
HC = H // P       # 6 feature chunks
NH = 12
DH = 64
FF = 3072
FC = FF // P      # 24
TS = S // P       # 16 key-token chunks
TQ = SQ // P      # 8 query-token chunks
NP_ = NH // 2     # 6 head pairs
VW = 162          # cols per pair in the V stationary layout
N_CORES = 8
EPS = 1e-5
BF16 = ml_dtypes.bfloat16

_CACHE = {}


def _ln_make(nc, mybir, pool, ps_pool, bc_pool, ones_row, eps_s, src_s, dst_s, w_s, b_s, ones_s, bc_tag="ops", plane_cb=None):
    """Feature-major LayerNorm over the partition (feature) axis, split so
    the per-plane stats matmuls can interleave with the producer loop.

    Returns (stats, finish).  Call stats(j) right after src plane j is
    written; call finish() after all planes.  Stats via ones-vector
    matmuls on the PE; mean/rstd broadcast across partitions with a
    ones-column PE matmul; the normalization DVE ops read the broadcast
    rows straight from PSUM (one PSUM operand per instruction).
    plane_cb(j), if given, runs after dst plane j is normalized.
    """
    f32 = mybir.dt.float32
    f32r = mybir.dt.float32r
    AF = mybir.ActivationFunctionType
    OP = mybir.AluOpType

    sum_ps = ps_pool.tile([1, SQ], f32, tag="lnsum", bufs=1)
    sq_ps = ps_pool.tile([1, SQ], f32, tag="lnsq", bufs=1)

    def stats(j):
        sq_t = pool.tile([P, SQ], f32r, tag="lnsqt", bufs=2)
        nc.vector.tensor_tensor(sq_t[:], src_s[:, j, :], src_s[:, j, :], OP.mult)
        for n in range(2):
            nc.tensor.matmul(
                sum_ps[:, n * 512 : (n + 1) * 512],
                lhsT=ones_s[:],
                rhs=src_s[:, j, n * 512 : (n + 1) * 512],
                start=(j == 0),
                stop=(j == HC - 1),
            )
            nc.tensor.matmul(
                sq_ps[:, n * 512 : (n + 1) * 512],
                lhsT=ones_s[:],
                rhs=sq_t[:, n * 512 : (n + 1) * 512],
                start=(j == 0),
                stop=(j == HC - 1),
            )

    def finish():
        mean = pool.tile([1, SQ], f32r, tag="lnmean", bufs=1)
        nc.vector.tensor_scalar_mul(mean[:], sum_ps[:], 1.0 / H)
        m2 = pool.tile([1, SQ], f32, tag="lntmp", bufs=2)
        nc.vector.tensor_tensor(m2[:], mean[:], mean[:], OP.mult)
        var = pool.tile([1, SQ], f32, tag="lntmp", bufs=2)
        nc.vector.scalar_tensor_tensor(
            out=var[:], in0=sq_ps[:], scalar=1.0 / H, in1=m2[:], op0=OP.mult,
            op1=OP.subtract,
        )
        std = pool.tile([1, SQ], f32, tag="lntmp", bufs=2)
        nc.scalar.activation(std[:], var[:], AF.Sqrt, bias=eps_s[:])
        rstd_f = pool.tile([1, SQ], f32, tag="lnrstdf", bufs=1)
        nc.vector.reciprocal_approx_fast(rstd_f[:], std[:])
        rstd = pool.tile([1, SQ], f32r, tag="lnrstd", bufs=1)
        nc.vector.tensor_copy(rstd[:], rstd_f[:])
        # broadcast mean/rstd across all 128 partitions
        mb_ps = bc_pool.tile([P, SQ], f32, tag=bc_tag)
        rb_ps = bc_pool.tile([P, SQ], f32, tag=bc_tag)
        for n in range(2):
            nc.tensor.matmul(
                mb_ps[:, n * 512 : (n + 1) * 512], lhsT=ones_row[:],
                rhs=mean[:, n * 512 : (n + 1) * 512], start=True, stop=True,
            )
            nc.tensor.matmul(
                rb_ps[:, n * 512 : (n + 1) * 512], lhsT=ones_row[:],
                rhs=rstd[:, n * 512 : (n + 1) * 512], start=True, stop=True,
            )
        for j in range(HC):
            t1 = pool.tile([P, SQ], f32, tag="lnt1", bufs=2)
            nc.vector.tensor_tensor(t1[:], src_s[:, j, :], mb_ps[:], OP.subtract)
            t2 = pool.tile([P, SQ], f32, tag="lnt2", bufs=2)
            nc.vector.scalar_tensor_tensor(
                out=t2[:], in0=t1[:], scalar=w_s[:, j : j + 1], in1=rb_ps[:],
                op0=OP.mult, op1=OP.mult,
            )
            nc.gpsimd.tensor_scalar(
                dst_s[:, j, :], t2[:], b_s[:, j : j + 1], None, OP.add,
            )
            if plane_cb is not None:
                plane_cb(j)

    return stats, finish


def _emit(nc, tc, t, mybir, make_identity):
    """Emit the per-core program. `t` maps tensor name -> DRAM AP."""
    from contextlib import ExitStack

    f32 = mybir.dt.float32
    f32r = mybir.dt.float32r
    bf16 = mybir.dt.bfloat16
    AF = mybir.ActivationFunctionType
    OP = mybir.AluOpType

    def mm(ps, lhsT, rhs, start, stop):
        nc.tensor.matmul(ps, lhsT=lhsT, rhs=rhs, start=start, stop=stop)

    with ExitStack() as ctx:
        aux = ctx.enter_context(tc.tile_pool(name="aux", bufs=1))

        def aux_load(name, shape, dtype=f32):
            tl = aux.tile(shape, dtype, tag=name)
            nc.gpsimd.dma_start(tl[:], t[name])
            return tl

        bq_s = aux_load("bq2", [P, HC])
        bk_s = aux_load("bk2", [P, HC])
        bo_s = aux_load("bo2", [P, HC])
        b2_s = aux_load("b22", [P, HC])
        l1w_s = aux_load("l1w", [P, HC])
        l1b_s = aux_load("l1b", [P, HC])
        l2w_s = aux_load("l2w", [P, HC])
        l2b_s = aux_load("l2b", [P, HC])
        b1_s = aux_load("b12", [P, FC])
        bvb_s = aux.tile([P, H], f32)
        nc.gpsimd.dma_start(bvb_s[:], t["bv"].partition_broadcast(P))
        ones_f = aux.tile([P, 1], f32)
        nc.vector.memset(ones_f[:], 1.0)
        ones_s = aux.tile([P, 1], f32r)
        nc.vector.tensor_copy(ones_s[:], ones_f[:])
        ones_rf = aux.tile([1, P], f32)
        nc.vector.memset(ones_rf[:], 1.0)
        ones_row = aux.tile([1, P], f32r)
        nc.vector.tensor_copy(ones_row[:], ones_rf[:])
        zero_s = aux.tile([P, 1], f32)
        nc.vector.memset(zero_s[:], 0.0)
        eps_s = aux.tile([1, 1], f32)
        nc.vector.memset(eps_s[:], EPS)
        ident_s = aux.tile([P, P], f32)
        make_identity(nc, ident_s[:])

        # x1 (LN1 output) outlives the attention/O-proj scopes below.
        keep = ctx.enter_context(tc.tile_pool(name="keep", bufs=1))
        x1_s = keep.tile([P, HC, SQ], f32r)

        with tc.tile_pool(name="resid", bufs=1) as resid:
            xTq_s = resid.tile([P, HC, SQ], f32r)
            for j in range(HC):
                eng = nc.sync if j % 2 == 0 else nc.scalar
                eng.dma_start(
                    xTq_s[:, j, :], t["xTq"][j * P : (j + 1) * P, :]
                )
            with tc.tile_pool(name="attn_out", bufs=1) as aop:
                attnT_s = aop.tile([P, HC, SQ], bf16)

                with tc.tile_pool(name="qkv_keep", bufs=1) as p2:
                    # qTz[p, h, q]: head h's 64 q-rows live at partitions
                    # (h%2)*64..+64 of plane h; the other 64 partitions stay
                    # zero so scores can contract over all 128 partitions.
                    qTz_s = p2.tile([P, NH, SQ], bf16)
                    nc.gpsimd.memset(qTz_s[:], 0.0)
                    kT_s = p2.tile([P, HC, S], bf16)
                    # v_s pair layout (stride VW=162):
                    #   [A.dims 0:64 | A.ones 64 | pad | B.ones 66 |
                    #    pad 67:98 | B.dims 98:162]
                    # Stationary window for A = cols base..base+128 (out
                    # rows 0:64 = A attn, row 64 = A softmax sums); for
                    # B = cols base+34..base+162 (out row 32 = B sums,
                    # rows 64:128 = B attn).  Partition-offset rules need
                    # the sums rows 32-aligned (64 for A, 32 for B).
                    v_s = p2.tile([P, TS, NP_ * VW], bf16)
                    v_view = v_s[:].rearrange("p t (c w) -> p t c w", w=VW)
                    nc.vector.memset(v_view[:, :, :, DH : DH + 3], 1.0)

                    # ---------------- QKV projections ----------------
                    with tc.tile_pool(name="qkvph", bufs=1) as ph, tc.tile_pool(
                        name="wstream", bufs=3
                    ) as ws, tc.tile_pool(
                        name="qkv_ps", bufs=3, space="PSUM"
                    ) as pp:
                        xT_s = ph.tile([P, HC, S], bf16)
                        xT_r = t["xT"].rearrange("(c p) s -> p c s", p=P)
                        for j in range(HC):
                            eng = nc.scalar if j % 2 == 0 else nc.sync
                            eng.dma_start(xT_s[:, j, :], xT_r[:, j, :])

                        # Q (our 1024 query tokens; fp32r from the residual copy)
                        for j in range(HC):
                            w_t = ws.tile([P, HC, P], f32r, tag="w")
                            nc.gpsimd.dma_start(
                                w_t[:],
                                t["Wq"][:, j * P : (j + 1) * P].rearrange(
                                    "(c p) m -> p c m", p=P
                                ),
                            )
                            ps = pp.tile([P, SQ], f32, tag="qkps")
                            for kc in range(HC):
                                for n in range(2):
                                    mm(
                                        ps[:, n * 512 : (n + 1) * 512],
                                        w_t[:, kc, :],
                                        xTq_s[:, kc, n * 512 : (n + 1) * 512],
                                        kc == 0,
                                        kc == HC - 1,
                                    )
                            nc.scalar.activation(
                                qTz_s[0:DH, 2 * j, :], ps[0:DH, :],
                                AF.Identity, bias=bq_s[0:DH, j : j + 1],
                            )
                            nc.scalar.activation(
                                qTz_s[DH:P, 2 * j + 1, :], ps[DH:P, :],
                                AF.Identity, bias=bq_s[DH:P, j : j + 1],
                            )

                        # K (all 2048 tokens, bf16)
                        for j in range(HC):
                            wk_t = ws.tile([P, HC, P], bf16, tag="w")
                            nc.gpsimd.dma_start(
                                wk_t[:],
                                t["Wk"][:, j * P : (j + 1) * P].rearrange(
                                    "(c p) m -> p c m", p=P
                                ),
                            )
                            for hf in range(2):
                                ps = pp.tile([P, SQ], f32, tag="qkps")
                                for kc in range(HC):
                                    for n in range(2):
                                        mm(
                                            ps[:, n * 512 : (n + 1) * 512],
                                            wk_t[:, kc, :],
                                            xT_s[
                                                :, kc,
                                                hf * SQ + n * 512 :
                                                hf * SQ + (n + 1) * 512,
                                            ],
                                            kc == 0,
                                            kc == HC - 1,
                                        )
                                nc.scalar.activation(
                                    kT_s[:, j, hf * SQ : (hf + 1) * SQ],
                                    ps[:],
                                    AF.Identity,
                                    bias=bk_s[:, j : j + 1],
                                )

                        # V (token-major, pair layout with ones columns)
                        wv_t = ws.tile([P, HC, H], bf16, tag="wv", bufs=1)
                        nc.gpsimd.dma_start(
                            wv_t[:], t["Wv"].rearrange("(c p) m -> p c m", p=P)
                        )
                        bvb_v = bvb_s[:].rearrange("p (c d) -> p c d", d=2 * DH)
                        for tt in range(TS):
                            ps = pp.tile([P, SQ], f32, tag="qkps")
                            for kc in range(HC):
                                mm(
                                    ps[:, 0:512],
                                    xT_s[:, kc, tt * P : (tt + 1) * P],
                                    wv_t[:, kc, 0:512],
                                    kc == 0,
                                    kc == HC - 1,
                                )
                                mm(
                                    ps[:, 512:H],
                                    xT_s[:, kc, tt * P : (tt + 1) * P],
                                    wv_t[:, kc, 512:H],
                                    kc == 0,
                                    kc == HC - 1,
                                )
                            ps_v = ps[:, 0:H].rearrange(
                                "p (c d) -> p c d", d=2 * DH
                            )
                            nc.vector.scalar_tensor_tensor(
                                out=v_view[:, tt, :, 0:DH],
                                in0=ps_v[:, :, 0:DH],
                                scalar=1.0,
                                in1=bvb_v[:, :, 0:DH],
                                op0=OP.mult,
                                op1=OP.add,
                            )
                            nc.vector.scalar_tensor_tensor(
                                out=v_view[:, tt, :, 98 : 98 + DH],
                                in0=ps_v[:, :, DH : 2 * DH],
                                scalar=1.0,
                                in1=bvb_v[:, :, DH : 2 * DH],
                                op0=OP.mult,
                                op1=OP.add,
                            )

                    # ---------------- attention ----------------
                    # Hybrid exp: the first 5 key-chunks of each head are
                    # exp'd directly PSUM->SBUF by Act; the remaining 11 are
                    # copied PSUM->SBUF bf16 by Vector and mega-exp'd in
                    # place by Act.  This balances the two engines that can
                    # read PSUM (~19us/head each).  AV for head h-1
                    # interleaves block-wise with scores of head h so exp
                    # latency never stalls the PE.
                    BLOCKS = [(0, 7, "direct"), (7, 10, "mega"),
                              (10, 13, "mega"), (13, 16, "mega")]
                    with tc.tile_pool(name="attn_sb", bufs=1) as ab, tc.tile_pool(
                        name="stage", bufs=3
                    ) as stp, tc.tile_pool(
                        name="prp", bufs=10
                    ) as prp, tc.tile_pool(
                        name="sc_ps", bufs=2, space="PSUM"
                    ) as pps, tc.tile_pool(
                        name="av_ps", bufs=2, space="PSUM"
                    ) as ppa:
                        stages = {}
                        avs = {}

                        def emit_scores_b(h, bi):
                            k0, k1, kind = BLOCKS[bi]
                            hc = h // 2
                            tiles = []
                            st = None
                            if kind == "mega":
                                st = stp.tile([P, 4, SQ], bf16, tag="st")
                            for kt in range(k0, k1):
                                sc = pps.tile([P, SQ], f32, tag="sc")
                                lhsT_k = kT_s[:, hc, kt * P : (kt + 1) * P]
                                for n in range(2):
                                    mm(
                                        sc[:, n * 512 : (n + 1) * 512],
                                        lhsT_k,
                                        qTz_s[:, h, n * 512 : (n + 1) * 512],
                                        True,
                                        True,
                                    )
                                if kind == "direct":
                                    pr = prp.tile([P, SQ], bf16, tag="pr")
                                    nc.scalar.activation(
                                        pr[:], sc[:], AF.Exp, bias=0.0,
                                        scale=0.125,
                                    )
                                    tiles.append(pr)
                                else:
                                    nc.vector.tensor_copy(
                                        st[:, kt - k0, :], sc[:]
                                    )
                            if kind == "mega":
                                nc.scalar.activation(
                                    st[:, 0 : k1 - k0, :], st[:, 0 : k1 - k0, :],
                                    AF.Exp, bias=0.0, scale=0.125,
                                )
                            stages[(h, bi)] = (kind, tiles, st)

                        def emit_av_b(h, bi):
                            k0, k1, _ = BLOCKS[bi]
                            hc, par = h // 2, h % 2
                            if bi == 0:
                                av = ppa.tile([P, SQ], f32, tag="av")
                                avs[h] = av
                            else:
                                av = avs[h]
                            kind, tiles, st = stages.pop((h, bi))
                            base = hc * VW + 34 * par
                            for kt in range(k0, k1):
                                pr = tiles[kt - k0] if kind == "direct" else st
                                prs = (
                                    pr[:, :] if kind == "direct"
                                    else pr[:, kt - k0, :]
                                )
                                for n in range(2):
                                    mm(
                                        av[:, n * 512 : (n + 1) * 512],
                                        v_s[:, kt, base : base + P],
                                        prs[:, n * 512 : (n + 1) * 512],
                                        kt == 0,
                                        kt == TS - 1,
                                    )

                        def emit_norm(h):
                            """Reciprocal of the softmax sums row (straight
                            from PSUM), partition-broadcast via ones-column
                            matmul, normalize into attnT."""
                            hc, par = h // 2, h % 2
                            av = avs.pop(h)
                            sp = ab.tile([P, SQ], bf16, tag="sp", bufs=1)
                            nc.vector.tensor_copy(sp[:], av[:])
                            sumrow = DH if par == 0 else 32
                            sv = ab.tile([1, SQ], f32, tag="sums", bufs=1)
                            nc.vector.tensor_copy(
                                sv[:], av[sumrow : sumrow + 1, :]
                            )
                            rec = ab.tile([1, SQ], f32, tag="rec", bufs=1)
                            nc.vector.reciprocal_approx_fast(rec[:], sv[:])
                            rec_r = ab.tile([1, SQ], f32r, tag="recr", bufs=1)
                            nc.vector.tensor_copy(rec_r[:], rec[:])
                            bc = ppa.tile([P, SQ], f32, tag="av")
                            for n in range(2):
                                mm(
                                    bc[:, n * 512 : (n + 1) * 512],
                                    ones_row[:],
                                    rec_r[:, n * 512 : (n + 1) * 512],
                                    True,
                                    True,
                                )
                            rows = slice(0, DH) if par == 0 else slice(DH, P)
                            nc.vector.tensor_tensor(
                                attnT_s[rows, hc, :], sp[rows, :], bc[rows, :],
                                OP.mult,
                            )

                        for h in range(NH):
                            for bi in range(len(BLOCKS)):
                                emit_scores_b(h, bi)
                                if h > 0:
                                    emit_av_b(h - 1, bi)
                            if h > 0:
                                emit_norm(h - 1)
                        for bi in range(len(BLOCKS)):
                            emit_av_b(NH - 1, bi)
                        emit_norm(NH - 1)

                # ---------------- O-projection + residual + LN1 ----------------
                with tc.tile_pool(name="oproj", bufs=1) as op_, tc.tile_pool(
                    name="wo_st", bufs=3
                ) as wop, tc.tile_pool(
                    name="o_ps", bufs=2, space="PSUM"
                ) as ppo, tc.tile_pool(
                    name="st_ps", bufs=1, space="PSUM"
                ) as ppst:
                    r1_s = op_.tile([P, HC, SQ], f32r)
                    ln1_stats, ln1_finish = _ln_make(
                        nc, mybir, op_, ppst, ppo, ones_row, eps_s,
                        r1_s, x1_s, l1w_s, l1b_s, ones_s,
                    )
                    for j in range(HC):
                        wo_t = wop.tile([P, HC, P], bf16, tag="wo")
                        nc.gpsimd.dma_start(
                            wo_t[:],
                            t["Wo"][:, j * P : (j + 1) * P].rearrange(
                                "(c p) m -> p c m", p=P
                            ),
                        )
                        ps = ppo.tile([P, SQ], f32, tag="ops")
                        for kc in range(HC):
                            for n in range(2):
                                mm(
                                    ps[:, n * 512 : (n + 1) * 512],
                                    wo_t[:, kc, :],
                                    attnT_s[
                                        :, kc, n * 512 : (n + 1) * 512
                                    ],
                                    kc == 0,
                                    kc == HC - 1,
                                )
                        nc.vector.scalar_tensor_tensor(
                            out=r1_s[:, j, :],
                            in0=ps[:],
                            scalar=bo_s[:, j : j + 1],
                            in1=xTq_s[:, j, :],
                            op0=OP.add,
                            op1=OP.add,
                        )
                        ln1_stats(j)
                    ln1_finish()

        # ---------------- MLP + LN2 + output ----------------
        # W2 runs kc-major in three j-pair passes; the first pass
        # accumulates each hT plane right behind its gelu, so W2's first
        # third is hidden under W1 and there is no W1->W2 barrier.
        with tc.tile_pool(name="mlp", bufs=1) as mp, tc.tile_pool(
            name="w1_st", bufs=3
        ) as w1p, tc.tile_pool(
            name="acc_ps", bufs=2, space="PSUM"
        ) as accp:
            hT_s = mp.tile([P, FC, SQ], bf16)
            r2_s = mp.tile([P, HC, SQ], f32r)
            w2_s = mp.tile([P, FC, H], bf16)
            w2_r = t["W2"].rearrange("(c p) m -> p c m", p=P)
            for kc in range(FC):
                eng = nc.scalar if kc % 2 == 0 else nc.sync
                eng.dma_start(w2_s[:, kc, :], w2_r[:, kc, :])

            def w2_terms(accs, jpair, m):
                for jj in range(2):
                    j = 2 * jpair + jj
                    for n in range(2):
                        mm(
                            accs[jj][:, n * 512 : (n + 1) * 512],
                            w2_s[:, m, j * P : (j + 1) * P],
                            hT_s[:, m, n * 512 : (n + 1) * 512],
                            m == 0,
                            m == FC - 1,
                        )

            def w2_emit_r2(accs, jpair):
                for jj in range(2):
                    j = 2 * jpair + jj
                    nc.vector.scalar_tensor_tensor(
                        out=r2_s[:, j, :],
                        in0=accs[jj][:],
                        scalar=b2_s[:, j : j + 1],
                        in1=x1_s[:, j, :],
                        op0=OP.add,
                        op1=OP.add,
                    )

            with tc.tile_pool(name="m_ps", bufs=2, space="PSUM") as ppm:
                acc_a = accp.tile([P, SQ], f32, tag="acc")
                acc_b = accp.tile([P, SQ], f32, tag="acc")
                accs0 = [acc_a, acc_b]
                for m in range(FC):
                    w1_t = w1p.tile([P, HC, P], f32r, tag="w1")
                    nc.gpsimd.dma_start(
                        w1_t[:],
                        t["W1"][:, m * P : (m + 1) * P].rearrange(
                            "(c p) n -> p c n", p=P
                        ),
                    )
                    ps = ppm.tile([P, SQ], f32, tag="mps")
                    for kc in range(HC):
                        for n in range(2):
                            mm(
                                ps[:, n * 512 : (n + 1) * 512],
                                w1_t[:, kc, :],
                                x1_s[:, kc, n * 512 : (n + 1) * 512],
                                kc == 0,
                                kc == HC - 1,
                            )
                    nc.scalar.activation(
                        hT_s[:, m, :], ps[:], AF.Gelu, bias=b1_s[:, m : m + 1]
                    )
                    if m > 0:
                        w2_terms(accs0, 0, m - 1)
                w2_terms(accs0, 0, FC - 1)
                w2_emit_r2(accs0, 0)

            with tc.tile_pool(name="st2_ps", bufs=1, space="PSUM") as ppst2:
                ln2_stats, ln2_finish = _ln_make(
                    nc, mybir, mp, ppst2, accp, ones_row, eps_s,
                    r2_s, r2_s, l2w_s, l2b_s, ones_s, bc_tag="acc",
                )
                ln2_stats(0)
                ln2_stats(1)
                for jpair in (1, 2):
                    acc_a = accp.tile([P, SQ], f32, tag="acc")
                    acc_b = accp.tile([P, SQ], f32, tag="acc")
                    accs = [acc_a, acc_b]
                    for m in range(FC):
                        w2_terms(accs, jpair, m)
                    w2_emit_r2(accs, jpair)
                    ln2_stats(2 * jpair)
                    ln2_stats(2 * jpair + 1)
                ln2_finish()

            # transpose back to token-major and store
            with tc.tile_pool(name="outp", bufs=2) as outp, tc.tile_pool(
                name="tr_ps", bufs=4, space="PSUM"
            ) as ppt:
                for tt in range(TQ):
                    out_t = outp.tile([P, H], f32, tag="out")
                    for j in range(HC):
                        tps = ppt.tile([P, P], f32, tag="tr")
                        nc.tensor.transpose(
                            tps[:],
                            r2_s[:, j, tt * P : (tt + 1) * P].bitcast(f32),
                            ident_s[:],
                        )
                        if j % 2 == 0:
                            nc.scalar.activation(
                                out_t[:, j * P : (j + 1) * P], tps[:],
                                AF.Identity, bias=zero_s[:],
                            )
                        else:
                            nc.vector.tensor_copy(
                                out_t[:, j * P : (j + 1) * P], tps[:]
                            )
                    nc.sync.dma_start(
                        t["y"][tt * P : (tt + 1) * P, :], out_t[:]
                    )


def _build():
    import concourse.bacc as bacc
    import concourse.tile as tile
    import concourse.mybir as mybir
    from concourse.masks import make_identity

    f32 = mybir.dt.float32
    f32r = mybir.dt.float32r
    bf16 = mybir.dt.bfloat16

    nc = bacc.Bacc(
        "TRN2", target_bir_lowering=False, debug=False, num_devices=N_CORES
    )
    specs = [
        ("xT", [H, S], bf16, "ExternalInput"),
        ("xTq", [H, SQ], f32r, "ExternalInput"),
        ("Wq", [H, H], f32r, "ExternalInput"),
        ("Wk", [H, H], bf16, "ExternalInput"),
        ("Wv", [H, H], bf16, "ExternalInput"),
        ("Wo", [H, H], bf16, "ExternalInput"),
        ("W1", [H, FF], f32r, "ExternalInput"),
        ("W2", [FF, H], bf16, "ExternalInput"),
        ("bq2", [P, HC], f32, "ExternalInput"),
        ("bk2", [P, HC], f32, "ExternalInput"),
        ("bv", [H], f32, "ExternalInput"),
        ("bo2", [P, HC], f32, "ExternalInput"),
        ("b12", [P, FC], f32, "ExternalInput"),
        ("b22", [P, HC], f32, "ExternalInput"),
        ("l1w", [P, HC], f32, "ExternalInput"),
        ("l1b", [P, HC], f32, "ExternalInput"),
        ("l2w", [P, HC], f32, "ExternalInput"),
        ("l2b", [P, HC], f32, "ExternalInput"),
        ("y", [SQ, H], f32, "ExternalOutput"),
    ]
    t = {
        name: nc.dram_tensor(name, shape, dt, kind=kind).ap()
        for name, shape, dt, kind in specs
    }
    with tile.TileContext(nc) as tc:
        _emit(nc, tc, t, mybir, make_identity)
    nc.compile()
    return nc


def _chunk_major(v):
    """[C*P] -> [P, C] with entry [p, c] = v[c*P + p]."""
    return np.ascontiguousarray(v.reshape(-1, P).T)


def prepare_in_maps(inputs):
    inp = {k: np.asarray(v) for k, v in inputs.items()}
    x = inp["x"].astype(np.float32)

    shared = {
        "Wq": inp["Wq"].astype(np.float32),
        "Wk": inp["Wk"].astype(BF16),
        "Wv": inp["Wv"].astype(BF16),
        "Wo": inp["Wo"].astype(BF16),
        "W1": inp["W1"].astype(np.float32),
        "W2": inp["W2"].astype(BF16),
        "bq2": _chunk_major(inp["bq"].astype(np.float32)),
        "bk2": _chunk_major(inp["bk"].astype(np.float32)),
        "bv": inp["bv"].astype(np.float32),
        "bo2": _chunk_major(inp["bo"].astype(np.float32)),
        "b12": _chunk_major(inp["b1"].astype(np.float32)),
        "b22": _chunk_major(inp["b2"].astype(np.float32)),
        "l1w": _chunk_major(inp["ln1_w"].astype(np.float32)),
        "l1b": _chunk_major(inp["ln1_b"].astype(np.float32)),
        "l2w": _chunk_major(inp["ln2_w"].astype(np.float32)),
        "l2b": _chunk_major(inp["ln2_b"].astype(np.float32)),
    }
    in_maps = []
    for c in range(N_CORES):
        b, hf = c // 2, c % 2
        xT = np.ascontiguousarray(x[b].T)
        m = dict(shared)
        m["xT"] = xT.astype(BF16)
        m["xTq"] = np.ascontiguousarray(xT[:, hf * SQ : (hf + 1) * SQ])
        in_maps.append(m)
    return in_maps


def get_program():
    if "nc" not in _CACHE:
        _CACHE["nc"] = _build()
    return _CACHE["nc"]


def kernel(**inputs):
    from concourse.bass_utils import run_bass_kernel_spmd

    nc = get_program()
    in_maps = prepare_in_maps(inputs)
    res = run_bass_kernel_spmd(nc, in_maps, core_ids=list(range(N_CORES)))
    out = np.empty((B, S, H), np.float32)
    for c in range(N_CORES):
        b, hf = c // 2, c % 2
        out[b, hf * SQ : (hf + 1) * SQ] = res.results[c]["y"]
    return out


# revision 37
# speedup vs baseline: 1.2317x; 1.2317x over previous
"""BertBlock kernel for 8 Trainium2 NeuronCores.

Sharding: pure data-parallel over (batch, half-sequence) tokens: core c
handles batch element c//2, query-token half c%2 (1024 tokens). Each core
recomputes K/V for the full 2048-token sequence of its batch element, so
no collectives are needed.

Device layout is feature-major ([feature, token]) end to end. The softmax
exp is the Act-engine bottleneck, so scores are staged: Pool/Vector copy
the PSUM score tiles to SBUF bf16 and the Act engine runs one wide exp per
half-head ([128, 8*1024], in place), amortizing the per-instruction
overhead 8x. Softmax denominators come from ones-columns in the V
stationary operand; the V layout per key-chunk pair is
[A.dims | A.ones | B.ones | B.dims] so the odd head's stationary window
(shifted by 2) lands its output rows at partitions 64:128 directly - no
partition-shift DMAs. Reciprocals use the fast approximate DVE op, and
the reciprocal row is partition-broadcast with a ones-column PE matmul.
AV for head h-1 is emitted after scores of head h so the exp latency
never stalls the PE.
"""

import numpy as np
import ml_dtypes

P = 128
B = 4
S = 2048          # sequence length (keys)
SQ = 1024         # query tokens per core
H = 768
HC = H // P       # 6 feature chunks
NH = 12
DH = 64
FF = 3072
FC = FF // P      # 24
TS = S // P       # 16 key-token chunks
TQ = SQ // P      # 8 query-token chunks
NP_ = NH // 2     # 6 head pairs
VW = 162          # cols per pair in the V stationary layout
N_CORES = 8
EPS = 1e-5
BF16 = ml_dtypes.bfloat16

_CACHE = {}


def _ln_make(nc, mybir, pool, ps_pool, bc_pool, ones_row, eps_s, src_s, dst_s, w_s, b_s, ones_s, bc_tag="ops", plane_cb=None):
    """Feature-major LayerNorm over the partition (feature) axis, split so
    the per-plane stats matmuls can interleave with the producer loop.

    Returns (stats, finish).  Call stats(j) right after src plane j is
    written; call finish() after all planes.  Stats via ones-vector
    matmuls on the PE; mean/rstd broadcast across partitions with a
    ones-column PE matmul; the normalization DVE ops read the broadcast
    rows straight from PSUM (one PSUM operand per instruction).
    plane_cb(j), if given, runs after dst plane j is normalized.
    """
    f32 = mybir.dt.float32
    f32r = mybir.dt.float32r
    AF = mybir.ActivationFunctionType
    OP = mybir.AluOpType

    sum_ps = ps_pool.tile([1, SQ], f32, tag="lnsum", bufs=1)
    sq_ps = ps_pool.tile([1, SQ], f32, tag="lnsq", bufs=1)

    def stats(j):
        sq_t = pool.tile([P, SQ], f32r, tag="lnsqt", bufs=2)
        nc.vector.tensor_tensor(sq_t[:], src_s[:, j, :], src_s[:, j, :], OP.mult)
        for n in range(2):
            nc.tensor.matmul(
                sum_ps[:, n * 512 : (n + 1) * 512],
                lhsT=ones_s[:],
                rhs=src_s[:, j, n * 512 : (n + 1) * 512],
                start=(j == 0),
                stop=(j == HC - 1),
            )
            nc.tensor.matmul(
                sq_ps[:, n * 512 : (n + 1) * 512],
                lhsT=ones_s[:],
                rhs=sq_t[:, n * 512 : (n + 1) * 512],
                start=(j == 0),
                stop=(j == HC - 1),
            )

    def finish():
        mean = pool.tile([1, SQ], f32r, tag="lnmean", bufs=1)
        nc.vector.tensor_scalar_mul(mean[:], sum_ps[:], 1.0 / H)
        m2 = pool.tile([1, SQ], f32, tag="lntmp", bufs=2)
        nc.vector.tensor_tensor(m2[:], mean[:], mean[:], OP.mult)
        var = pool.tile([1, SQ], f32, tag="lntmp", bufs=2)
        nc.vector.scalar_tensor_tensor(
            out=var[:], in0=sq_ps[:], scalar=1.0 / H, in1=m2[:], op0=OP.mult,
            op1=OP.subtract,
        )
        std = pool.tile([1, SQ], f32, tag="lntmp", bufs=2)
        nc.scalar.activation(std[:], var[:], AF.Sqrt, bias=eps_s[:])
        rstd_f = pool.tile([1, SQ], f32, tag="lnrstdf", bufs=1)
        nc.vector.reciprocal_approx_fast(rstd_f[:], std[:])
        rstd = pool.tile([1, SQ], f32r, tag="lnrstd", bufs=1)
        nc.vector.tensor_copy(rstd[:], rstd_f[:])
        # broadcast mean/rstd across all 128 partitions
        mb_ps = bc_pool.tile([P, SQ], f32, tag=bc_tag)
        rb_ps = bc_pool.tile([P, SQ], f32, tag=bc_tag)
        for n in range(2):
            nc.tensor.matmul(
                mb_ps[:, n * 512 : (n + 1) * 512], lhsT=ones_row[:],
                rhs=mean[:, n * 512 : (n + 1) * 512], start=True, stop=True,
            )
            nc.tensor.matmul(
                rb_ps[:, n * 512 : (n + 1) * 512], lhsT=ones_row[:],
                rhs=rstd[:, n * 512 : (n + 1) * 512], start=True, stop=True,
            )
        for j in range(HC):
            t1 = pool.tile([P, SQ], f32, tag="lnt1", bufs=2)
            nc.vector.tensor_tensor(t1[:], src_s[:, j, :], mb_ps[:], OP.subtract)
            t2 = pool.tile([P, SQ], f32, tag="lnt2", bufs=2)
            nc.vector.scalar_tensor_tensor(
                out=t2[:], in0=t1[:], scalar=w_s[:, j : j + 1], in1=rb_ps[:],
                op0=OP.mult, op1=OP.mult,
            )
            nc.vector.tensor_scalar(
                dst_s[:, j, :], t2[:], b_s[:, j : j + 1], None, OP.add,
            )
            if plane_cb is not None:
                plane_cb(j)

    return stats, finish


def _emit(nc, tc, t, mybir, make_identity):
    """Emit the per-core program. `t` maps tensor name -> DRAM AP."""
    from contextlib import ExitStack

    f32 = mybir.dt.float32
    f32r = mybir.dt.float32r
    bf16 = mybir.dt.bfloat16
    AF = mybir.ActivationFunctionType
    OP = mybir.AluOpType

    def mm(ps, lhsT, rhs, start, stop):
        nc.tensor.matmul(ps, lhsT=lhsT, rhs=rhs, start=start, stop=stop)

    with ExitStack() as ctx:
        aux = ctx.enter_context(tc.tile_pool(name="aux", bufs=1))

        def aux_load(name, shape, dtype=f32):
            tl = aux.tile(shape, dtype, tag=name)
            nc.gpsimd.dma_start(tl[:], t[name])
            return tl

        bq_s = aux_load("bq2", [P, HC])
        bk_s = aux_load("bk2", [P, HC])
        bo_s = aux_load("bo2", [P, HC])
        b2_s = aux_load("b22", [P, HC])
        l1w_s = aux_load("l1w", [P, HC])
        l1b_s = aux_load("l1b", [P, HC])
        l2w_s = aux_load("l2w", [P, HC])
        l2b_s = aux_load("l2b", [P, HC])
        b1_s = aux_load("b12", [P, FC])
        bvb_s = aux.tile([P, H], f32)
        nc.gpsimd.dma_start(bvb_s[:], t["bv"].partition_broadcast(P))
        ones_f = aux.tile([P, 1], f32)
        nc.vector.memset(ones_f[:], 1.0)
        ones_s = aux.tile([P, 1], f32r)
        nc.vector.tensor_copy(ones_s[:], ones_f[:])
        ones_rf = aux.tile([1, P], f32)
        nc.vector.memset(ones_rf[:], 1.0)
        ones_row = aux.tile([1, P], f32r)
        nc.vector.tensor_copy(ones_row[:], ones_rf[:])
        zero_s = aux.tile([P, 1], f32)
        nc.vector.memset(zero_s[:], 0.0)
        eps_s = aux.tile([1, 1], f32)
        nc.vector.memset(eps_s[:], EPS)
        ident_s = aux.tile([P, P], f32)
        make_identity(nc, ident_s[:])

        # x1 (LN1 output) outlives the attention/O-proj scopes below.
        keep = ctx.enter_context(tc.tile_pool(name="keep", bufs=1))
        x1_s = keep.tile([P, HC, SQ], f32r)

        with tc.tile_pool(name="resid", bufs=1) as resid:
            xTq_s = resid.tile([P, HC, SQ], f32r)
            for j in range(HC):
                eng = nc.sync if j % 2 == 0 else nc.scalar
                eng.dma_start(
                    xTq_s[:, j, :], t["xTq"][j * P : (j + 1) * P, :]
                )
            with tc.tile_pool(name="attn_out", bufs=1) as aop:
                attnT_s = aop.tile([P, HC, SQ], bf16)

                with tc.tile_pool(name="qkv_keep", bufs=1) as p2:
                    # qTz[p, h, q]: head h's 64 q-rows live at partitions
                    # (h%2)*64..+64 of plane h; the other 64 partitions stay
                    # zero so scores can contract over all 128 partitions.
                    qTz_s = p2.tile([P, NH, SQ], bf16)
                    nc.gpsimd.memset(qTz_s[:], 0.0)
                    kT_s = p2.tile([P, HC, S], bf16)
                    # v_s pair layout (stride VW=162):
                    #   [A.dims 0:64 | A.ones 64 | pad | B.ones 66 |
                    #    pad 67:98 | B.dims 98:162]
                    # Stationary window for A = cols base..base+128 (out
                    # rows 0:64 = A attn, row 64 = A softmax sums); for
                    # B = cols base+34..base+162 (out row 32 = B sums,
                    # rows 64:128 = B attn).  Partition-offset rules need
                    # the sums rows 32-aligned (64 for A, 32 for B).
                    v_s = p2.tile([P, TS, NP_ * VW], bf16)
                    v_view = v_s[:].rearrange("p t (c w) -> p t c w", w=VW)
                    nc.vector.memset(v_view[:, :, :, DH : DH + 3], 1.0)

                    # ---------------- QKV projections ----------------
                    with tc.tile_pool(name="qkvph", bufs=1) as ph, tc.tile_pool(
                        name="wstream", bufs=3
                    ) as ws, tc.tile_pool(
                        name="qkv_ps", bufs=3, space="PSUM"
                    ) as pp:
                        xT_s = ph.tile([P, HC, S], bf16)
                        xT_r = t["xT"].rearrange("(c p) s -> p c s", p=P)
                        for j in range(HC):
                            eng = nc.scalar if j % 2 == 0 else nc.sync
                            eng.dma_start(xT_s[:, j, :], xT_r[:, j, :])

                        # Q (our 1024 query tokens; fp32r from the residual copy)
                        for j in range(HC):
                            w_t = ws.tile([P, HC, P], f32r, tag="w")
                            nc.gpsimd.dma_start(
                                w_t[:],
                                t["Wq"][:, j * P : (j + 1) * P].rearrange(
                                    "(c p) m -> p c m", p=P
                                ),
                            )
                            ps = pp.tile([P, SQ], f32, tag="qkps")
                            for kc in range(HC):
                                for n in range(2):
                                    mm(
                                        ps[:, n * 512 : (n + 1) * 512],
                                        w_t[:, kc, :],
                                        xTq_s[:, kc, n * 512 : (n + 1) * 512],
                                        kc == 0,
                                        kc == HC - 1,
                                    )
                            nc.scalar.activation(
                                qTz_s[0:DH, 2 * j, :], ps[0:DH, :],
                                AF.Identity, bias=bq_s[0:DH, j : j + 1],
                            )
                            nc.scalar.activation(
                                qTz_s[DH:P, 2 * j + 1, :], ps[DH:P, :],
                                AF.Identity, bias=bq_s[DH:P, j : j + 1],
                            )

                        # K (all 2048 tokens, bf16)
                        for j in range(HC):
                            wk_t = ws.tile([P, HC, P], bf16, tag="w")
                            nc.gpsimd.dma_start(
                                wk_t[:],
                                t["Wk"][:, j * P : (j + 1) * P].rearrange(
                                    "(c p) m -> p c m", p=P
                                ),
                            )
                            for hf in range(2):
                                ps = pp.tile([P, SQ], f32, tag="qkps")
                                for kc in range(HC):
                                    for n in range(2):
                                        mm(
                                            ps[:, n * 512 : (n + 1) * 512],
                                            wk_t[:, kc, :],
                                            xT_s[
                                                :, kc,
                                                hf * SQ + n * 512 :
                                                hf * SQ + (n + 1) * 512,
                                            ],
                                            kc == 0,
                                            kc == HC - 1,
                                        )
                                nc.scalar.activation(
                                    kT_s[:, j, hf * SQ : (hf + 1) * SQ],
                                    ps[:],
                                    AF.Identity,
                                    bias=bk_s[:, j : j + 1],
                                )

                        # V (token-major, pair layout with ones columns)
                        wv_t = ws.tile([P, HC, H], bf16, tag="wv", bufs=1)
                        nc.gpsimd.dma_start(
                            wv_t[:], t["Wv"].rearrange("(c p) m -> p c m", p=P)
                        )
                        bvb_v = bvb_s[:].rearrange("p (c d) -> p c d", d=2 * DH)
                        for tt in range(TS):
                            ps = pp.tile([P, SQ], f32, tag="qkps")
                            for kc in range(HC):
                                mm(
                                    ps[:, 0:512],
                                    xT_s[:, kc, tt * P : (tt + 1) * P],
                                    wv_t[:, kc, 0:512],
                                    kc == 0,
                                    kc == HC - 1,
                                )
                                mm(
                                    ps[:, 512:H],
                                    xT_s[:, kc, tt * P : (tt + 1) * P],
                                    wv_t[:, kc, 512:H],
                                    kc == 0,
                                    kc == HC - 1,
                                )
                            ps_v = ps[:, 0:H].rearrange(
                                "p (c d) -> p c d", d=2 * DH
                            )
                            nc.vector.scalar_tensor_tensor(
                                out=v_view[:, tt, :, 0:DH],
                                in0=ps_v[:, :, 0:DH],
                                scalar=1.0,
                                in1=bvb_v[:, :, 0:DH],
                                op0=OP.mult,
                                op1=OP.add,
                            )
                            nc.vector.scalar_tensor_tensor(
                                out=v_view[:, tt, :, 98 : 98 + DH],
                                in0=ps_v[:, :, DH : 2 * DH],
                                scalar=1.0,
                                in1=bvb_v[:, :, DH : 2 * DH],
                                op0=OP.mult,
                                op1=OP.add,
                            )

                    # ---------------- attention ----------------
                    # Hybrid exp: the first 5 key-chunks of each head are
                    # exp'd directly PSUM->SBUF by Act; the remaining 11 are
                    # copied PSUM->SBUF bf16 by Vector and mega-exp'd in
                    # place by Act.  This balances the two engines that can
                    # read PSUM (~19us/head each).  AV for head h-1
                    # interleaves block-wise with scores of head h so exp
                    # latency never stalls the PE.
                    BLOCKS = [(0, 7, "direct"), (7, 10, "mega"),
                              (10, 13, "mega"), (13, 16, "mega")]
                    with tc.tile_pool(name="attn_sb", bufs=1) as ab, tc.tile_pool(
                        name="stage", bufs=3
                    ) as stp, tc.tile_pool(
                        name="prp", bufs=10
                    ) as prp, tc.tile_pool(
                        name="sc_ps", bufs=2, space="PSUM"
                    ) as pps, tc.tile_pool(
                        name="av_ps", bufs=2, space="PSUM"
                    ) as ppa:
                        stages = {}
                        avs = {}

                        def emit_scores_b(h, bi):
                            k0, k1, kind = BLOCKS[bi]
                            hc = h // 2
                            tiles = []
                            st = None
                            if kind == "mega":
                                st = stp.tile([P, 4, SQ], bf16, tag="st")
                            for kt in range(k0, k1):
                                sc = pps.tile([P, SQ], f32, tag="sc")
                                lhsT_k = kT_s[:, hc, kt * P : (kt + 1) * P]
                                for n in range(2):
                                    mm(
                                        sc[:, n * 512 : (n + 1) * 512],
                                        lhsT_k,
                                        qTz_s[:, h, n * 512 : (n + 1) * 512],
                                        True,
                                        True,
                                    )
                                if kind == "direct":
                                    pr = prp.tile([P, SQ], bf16, tag="pr")
                                    nc.scalar.activation(
                                        pr[:], sc[:], AF.Exp, bias=0.0,
                                        scale=0.125,
                                    )
                                    tiles.append(pr)
                                else:
                                    nc.vector.tensor_copy(
                                        st[:, kt - k0, :], sc[:]
                                    )
                            if kind == "mega":
                                nc.scalar.activation(
                                    st[:, 0 : k1 - k0, :], st[:, 0 : k1 - k0, :],
                                    AF.Exp, bias=0.0, scale=0.125,
                                )
                            stages[(h, bi)] = (kind, tiles, st)

                        def emit_av_b(h, bi):
                            k0, k1, _ = BLOCKS[bi]
                            hc, par = h // 2, h % 2
                            if bi == 0:
                                av = ppa.tile([P, SQ], f32, tag="av")
                                avs[h] = av
                            else:
                                av = avs[h]
                            kind, tiles, st = stages.pop((h, bi))
                            base = hc * VW + 34 * par
                            for kt in range(k0, k1):
                                pr = tiles[kt - k0] if kind == "direct" else st
                                prs = (
                                    pr[:, :] if kind == "direct"
                                    else pr[:, kt - k0, :]
                                )
                                for n in range(2):
                                    mm(
                                        av[:, n * 512 : (n + 1) * 512],
                                        v_s[:, kt, base : base + P],
                                        prs[:, n * 512 : (n + 1) * 512],
                                        kt == 0,
                                        kt == TS - 1,
                                    )

                        def emit_norm(h):
                            """Reciprocal of the softmax sums row (straight
                            from PSUM), partition-broadcast via ones-column
                            matmul, normalize into attnT."""
                            hc, par = h // 2, h % 2
                            av = avs.pop(h)
                            sp = ab.tile([P, SQ], bf16, tag="sp", bufs=1)
                            nc.vector.tensor_copy(sp[:], av[:])
                            sumrow = DH if par == 0 else 32
                            sv = ab.tile([1, SQ], f32, tag="sums", bufs=1)
                            nc.vector.tensor_copy(
                                sv[:], av[sumrow : sumrow + 1, :]
                            )
                            rec = ab.tile([1, SQ], f32, tag="rec", bufs=1)
                            nc.vector.reciprocal_approx_fast(rec[:], sv[:])
                            rec_r = ab.tile([1, SQ], f32r, tag="recr", bufs=1)
                            nc.vector.tensor_copy(rec_r[:], rec[:])
                            bc = ppa.tile([P, SQ], f32, tag="av")
                            for n in range(2):
                                mm(
                                    bc[:, n * 512 : (n + 1) * 512],
                                    ones_row[:],
                                    rec_r[:, n * 512 : (n + 1) * 512],
                                    True,
                                    True,
                                )
                            rows = slice(0, DH) if par == 0 else slice(DH, P)
                            nc.vector.tensor_tensor(
                                attnT_s[rows, hc, :], sp[rows, :], bc[rows, :],
                                OP.mult,
                            )

                        for h in range(NH):
                            for bi in range(len(BLOCKS)):
                                emit_scores_b(h, bi)
                                if h > 0:
                                    emit_av_b(h - 1, bi)
                            if h > 0:
                                emit_norm(h - 1)
                        for bi in range(len(BLOCKS)):
                            emit_av_b(NH - 1, bi)
                        emit_norm(NH - 1)

                # ---------------- O-projection + residual + LN1 ----------------
                with tc.tile_pool(name="oproj", bufs=1) as op_, tc.tile_pool(
                    name="wo_st", bufs=3
                ) as wop, tc.tile_pool(
                    name="o_ps", bufs=2, space="PSUM"
                ) as ppo, tc.tile_pool(
                    name="st_ps", bufs=1, space="PSUM"
                ) as ppst:
                    r1_s = op_.tile([P, HC, SQ], f32r)
                    ln1_stats, ln1_finish = _ln_make(
                        nc, mybir, op_, ppst, ppo, ones_row, eps_s,
                        r1_s, x1_s, l1w_s, l1b_s, ones_s,
                    )
                    for j in range(HC):
                        wo_t = wop.tile([P, HC, P], bf16, tag="wo")
                        nc.gpsimd.dma_start(
                            wo_t[:],
                            t["Wo"][:, j * P : (j + 1) * P].rearrange(
                                "(c p) m -> p c m", p=P
                            ),
                        )
                        ps = ppo.tile([P, SQ], f32, tag="ops")
                        for kc in range(HC):
                            for n in range(2):
                                mm(
                                    ps[:, n * 512 : (n + 1) * 512],
                                    wo_t[:, kc, :],
                                    attnT_s[
                                        :, kc, n * 512 : (n + 1) * 512
                                    ],
                                    kc == 0,
                                    kc == HC - 1,
                                )
                        nc.vector.scalar_tensor_tensor(
                            out=r1_s[:, j, :],
                            in0=ps[:],
                            scalar=bo_s[:, j : j + 1],
                            in1=xTq_s[:, j, :],
                            op0=OP.add,
                            op1=OP.add,
                        )
                        ln1_stats(j)
                    ln1_finish()

        # ---------------- MLP + LN2 + output ----------------
        # W2 runs kc-major in three j-pair passes; the first pass
        # accumulates each hT plane right behind its gelu, so W2's first
        # third is hidden under W1 and there is no W1->W2 barrier.
        with tc.tile_pool(name="mlp", bufs=1) as mp, tc.tile_pool(
            name="w1_st", bufs=3
        ) as w1p, tc.tile_pool(
            name="acc_ps", bufs=2, space="PSUM"
        ) as accp:
            hT_s = mp.tile([P, FC, SQ], bf16)
            r2_s = mp.tile([P, HC, SQ], f32r)
            w2_s = mp.tile([P, FC, H], bf16)
            w2_r = t["W2"].rearrange("(c p) m -> p c m", p=P)
            for kc in range(FC):
                eng = nc.scalar if kc % 2 == 0 else nc.sync
                eng.dma_start(w2_s[:, kc, :], w2_r[:, kc, :])

            def w2_terms(accs, jpair, m):
                for jj in range(2):
                    j = 2 * jpair + jj
                    for n in range(2):
                        mm(
                            accs[jj][:, n * 512 : (n + 1) * 512],
                            w2_s[:, m, j * P : (j + 1) * P],
                            hT_s[:, m, n * 512 : (n + 1) * 512],
                            m == 0,
                            m == FC - 1,
                        )

            def w2_emit_r2(accs, jpair):
                for jj in range(2):
                    j = 2 * jpair + jj
                    nc.vector.scalar_tensor_tensor(
                        out=r2_s[:, j, :],
                        in0=accs[jj][:],
                        scalar=b2_s[:, j : j + 1],
                        in1=x1_s[:, j, :],
                        op0=OP.add,
                        op1=OP.add,
                    )

            with tc.tile_pool(name="m_ps", bufs=2, space="PSUM") as ppm:
                acc_a = accp.tile([P, SQ], f32, tag="acc")
                acc_b = accp.tile([P, SQ], f32, tag="acc")
                accs0 = [acc_a, acc_b]
                for m in range(FC):
                    w1_t = w1p.tile([P, HC, P], f32r, tag="w1")
                    nc.gpsimd.dma_start(
                        w1_t[:],
                        t["W1"][:, m * P : (m + 1) * P].rearrange(
                            "(c p) n -> p c n", p=P
                        ),
                    )
                    ps = ppm.tile([P, SQ], f32, tag="mps")
                    for kc in range(HC):
                        for n in range(2):
                            mm(
                                ps[:, n * 512 : (n + 1) * 512],
                                w1_t[:, kc, :],
                                x1_s[:, kc, n * 512 : (n + 1) * 512],
                                kc == 0,
                                kc == HC - 1,
                            )
                    nc.scalar.activation(
                        hT_s[:, m, :], ps[:], AF.Gelu, bias=b1_s[:, m : m + 1]
                    )
                    if m > 0:
                        w2_terms(accs0, 0, m - 1)
                w2_terms(accs0, 0, FC - 1)
                w2_emit_r2(accs0, 0)

            with tc.tile_pool(name="st2_ps", bufs=1, space="PSUM") as ppst2:
                ln2_stats, ln2_finish = _ln_make(
                    nc, mybir, mp, ppst2, accp, ones_row, eps_s,
                    r2_s, r2_s, l2w_s, l2b_s, ones_s, bc_tag="acc",
                )
                ln2_stats(0)
                ln2_stats(1)
                for jpair in (1, 2):
                    acc_a = accp.tile([P, SQ], f32, tag="acc")
                    acc_b = accp.tile([P, SQ], f32, tag="acc")
                    accs = [acc_a, acc_b]
                    for m in range(FC):
                        w2_terms(accs, jpair, m)
                    w2_emit_r2(accs, jpair)
                    ln2_stats(2 * jpair)
                    ln2_stats(2 * jpair + 1)
                ln2_finish()

            # transpose back to token-major and store
            with tc.tile_pool(name="outp", bufs=2) as outp, tc.tile_pool(
                name="tr_ps", bufs=4, space="PSUM"
            ) as ppt:
                for tt in range(TQ):
                    out_t = outp.tile([P, H], f32, tag="out")
                    for j in range(HC):
                        tps = ppt.tile([P, P], f32, tag="tr")
                        nc.tensor.transpose(
                            tps[:],
                            r2_s[:, j, tt * P : (tt + 1) * P].bitcast(f32),
                            ident_s[:],
                        )
                        if j % 2 == 0:
                            nc.scalar.activation(
                                out_t[:, j * P : (j + 1) * P], tps[:],
                                AF.Identity, bias=zero_s[:],
                            )
                        else:
                            nc.vector.tensor_copy(
                                out_t[:, j * P : (j + 1) * P], tps[:]
                            )
                    nc.sync.dma_start(
                        t["y"][tt * P : (tt + 1) * P, :], out_t[:]
                    )


def _build():
    import concourse.bacc as bacc
    import concourse.tile as tile
    import concourse.mybir as mybir
    from concourse.masks import make_identity

    f32 = mybir.dt.float32
    f32r = mybir.dt.float32r
    bf16 = mybir.dt.bfloat16

    nc = bacc.Bacc(
        "TRN2", target_bir_lowering=False, debug=False, num_devices=N_CORES
    )
    specs = [
        ("xT", [H, S], bf16, "ExternalInput"),
        ("xTq", [H, SQ], f32r, "ExternalInput"),
        ("Wq", [H, H], f32r, "ExternalInput"),
        ("Wk", [H, H], bf16, "ExternalInput"),
        ("Wv", [H, H], bf16, "ExternalInput"),
        ("Wo", [H, H], bf16, "ExternalInput"),
        ("W1", [H, FF], f32r, "ExternalInput"),
        ("W2", [FF, H], bf16, "ExternalInput"),
        ("bq2", [P, HC], f32, "ExternalInput"),
        ("bk2", [P, HC], f32, "ExternalInput"),
        ("bv", [H], f32, "ExternalInput"),
        ("bo2", [P, HC], f32, "ExternalInput"),
        ("b12", [P, FC], f32, "ExternalInput"),
        ("b22", [P, HC], f32, "ExternalInput"),
        ("l1w", [P, HC], f32, "ExternalInput"),
        ("l1b", [P, HC], f32, "ExternalInput"),
        ("l2w", [P, HC], f32, "ExternalInput"),
        ("l2b", [P, HC], f32, "ExternalInput"),
        ("y", [SQ, H], f32, "ExternalOutput"),
    ]
    t = {
        name: nc.dram_tensor(name, shape, dt, kind=kind).ap()
        for name, shape, dt, kind in specs
    }
    with tile.TileContext(nc) as tc:
        _emit(nc, tc, t, mybir, make_identity)
    nc.compile()
    return nc


def _chunk_major(v):
    """[C*P] -> [P, C] with entry [p, c] = v[c*P + p]."""
    return np.ascontiguousarray(v.reshape(-1, P).T)


def prepare_in_maps(inputs):
    inp = {k: np.asarray(v) for k, v in inputs.items()}
    x = inp["x"].astype(np.float32)

    shared = {
        "Wq": inp["Wq"].astype(np.float32),
        "Wk": inp["Wk"].astype(BF16),
        "Wv": inp["Wv"].astype(BF16),
        "Wo": inp["Wo"].astype(BF16),
        "W1": inp["W1"].astype(np.float32),
        "W2": inp["W2"].astype(BF16),
        "bq2": _chunk_major(inp["bq"].astype(np.float32)),
        "bk2": _chunk_major(inp["bk"].astype(np.float32)),
        "bv": inp["bv"].astype(np.float32),
        "bo2": _chunk_major(inp["bo"].astype(np.float32)),
        "b12": _chunk_major(inp["b1"].astype(np.float32)),
        "b22": _chunk_major(inp["b2"].astype(np.float32)),
        "l1w": _chunk_major(inp["ln1_w"].astype(np.float32)),
        "l1b": _chunk_major(inp["ln1_b"].astype(np.float32)),
        "l2w": _chunk_major(inp["ln2_w"].astype(np.float32)),
        "l2b": _chunk_major(inp["ln2_b"].astype(np.float32)),
    }
    in_maps = []
    for c in range(N_CORES):
        b, hf = c // 2, c % 2
        xT = np.ascontiguousarray(x[b].T)
        m = dict(shared)
        m["xT"] = xT.astype(BF16)
        m["xTq"] = np.ascontiguousarray(xT[:, hf * SQ : (hf + 1) * SQ])
        in_maps.append(m)
    return in_maps


def get_program():
    if "nc" not in _CACHE:
        _CACHE["nc"] = _build()
    return _CACHE["nc"]


def kernel(**inputs):
    from concourse.bass_utils import run_bass_kernel_spmd

    nc = get_program()
    in_maps = prepare_in_maps(inputs)
    res = run_bass_kernel_spmd(nc, in_maps, core_ids=list(range(N_CORES)))
    out = np.empty((B, S, H), np.float32)
    for c in range(N_CORES):
        b, hf = c // 2, c % 2
        out[b, hf * SQ : (hf + 1) * SQ] = res.results[c]["y"]
    return out


# revision 39
# speedup vs baseline: 1.2495x; 1.0145x over previous
"""BertBlock kernel for 8 Trainium2 NeuronCores.

Sharding: pure data-parallel over (batch, half-sequence) tokens: core c
handles batch element c//2, query-token half c%2 (1024 tokens). Each core
recomputes K/V for the full 2048-token sequence of its batch element, so
no collectives are needed.

Device layout is feature-major ([feature, token]) end to end. The softmax
exp is the Act-engine bottleneck, so scores are staged: Pool/Vector copy
the PSUM score tiles to SBUF bf16 and the Act engine runs one wide exp per
half-head ([128, 8*1024], in place), amortizing the per-instruction
overhead 8x. Softmax denominators come from ones-columns in the V
stationary operand; the V layout per key-chunk pair is
[A.dims | A.ones | B.ones | B.dims] so the odd head's stationary window
(shifted by 2) lands its output rows at partitions 64:128 directly - no
partition-shift DMAs. Reciprocals use the fast approximate DVE op, and
the reciprocal row is partition-broadcast with a ones-column PE matmul.
AV for head h-1 is emitted after scores of head h so the exp latency
never stalls the PE.
"""

import numpy as np
import ml_dtypes

P = 128
B = 4
S = 2048          # sequence length (keys)
SQ = 1024         # query tokens per core
H = 768
HC = H // P       # 6 feature chunks
NH = 12
DH = 64
FF = 3072
FC = FF // P      # 24
TS = S // P       # 16 key-token chunks
TQ = SQ // P      # 8 query-token chunks
NP_ = NH // 2     # 6 head pairs
VW = 162          # cols per pair in the V stationary layout
N_CORES = 8
EPS = 1e-5
BF16 = ml_dtypes.bfloat16

_CACHE = {}


def _ln_make(nc, mybir, pool, ps_pool, bc_pool, ones_row, eps_s, src_s, dst_s, w_s, b_s, ones_s, bc_tag="ops", plane_cb=None):
    """Feature-major LayerNorm over the partition (feature) axis, split so
    the per-plane stats matmuls can interleave with the producer loop.

    Returns (stats, finish).  Call stats(j) right after src plane j is
    written; call finish() after all planes.  Stats via ones-vector
    matmuls on the PE; mean/rstd broadcast across partitions with a
    ones-column PE matmul; the normalization DVE ops read the broadcast
    rows straight from PSUM (one PSUM operand per instruction).
    plane_cb(j), if given, runs after dst plane j is normalized.
    """
    f32 = mybir.dt.float32
    f32r = mybir.dt.float32r
    AF = mybir.ActivationFunctionType
    OP = mybir.AluOpType

    sum_ps = ps_pool.tile([1, SQ], f32, tag="lnsum", bufs=1)
    sq_ps = ps_pool.tile([1, SQ], f32, tag="lnsq", bufs=1)

    def stats(j):
        sq_t = pool.tile([P, SQ], f32r, tag="lnsqt", bufs=2)
        nc.vector.tensor_tensor(sq_t[:], src_s[:, j, :], src_s[:, j, :], OP.mult)
        for n in range(2):
            nc.tensor.matmul(
                sum_ps[:, n * 512 : (n + 1) * 512],
                lhsT=ones_s[:],
                rhs=src_s[:, j, n * 512 : (n + 1) * 512],
                start=(j == 0),
                stop=(j == HC - 1),
            )
            nc.tensor.matmul(
                sq_ps[:, n * 512 : (n + 1) * 512],
                lhsT=ones_s[:],
                rhs=sq_t[:, n * 512 : (n + 1) * 512],
                start=(j == 0),
                stop=(j == HC - 1),
            )

    def finish():
        mean = pool.tile([1, SQ], f32r, tag="lnmean", bufs=1)
        nc.vector.tensor_scalar_mul(mean[:], sum_ps[:], 1.0 / H)
        m2 = pool.tile([1, SQ], f32, tag="lntmp", bufs=2)
        nc.vector.tensor_tensor(m2[:], mean[:], mean[:], OP.mult)
        var = pool.tile([1, SQ], f32, tag="lntmp", bufs=2)
        nc.vector.scalar_tensor_tensor(
            out=var[:], in0=sq_ps[:], scalar=1.0 / H, in1=m2[:], op0=OP.mult,
            op1=OP.subtract,
        )
        std = pool.tile([1, SQ], f32, tag="lntmp", bufs=2)
        nc.scalar.activation(std[:], var[:], AF.Sqrt, bias=eps_s[:])
        rstd_f = pool.tile([1, SQ], f32, tag="lnrstdf", bufs=1)
        nc.vector.reciprocal_approx_fast(rstd_f[:], std[:])
        rstd = pool.tile([1, SQ], f32r, tag="lnrstd", bufs=1)
        nc.vector.tensor_copy(rstd[:], rstd_f[:])
        # broadcast mean/rstd across all 128 partitions
        mb_ps = bc_pool.tile([P, SQ], f32, tag=bc_tag)
        rb_ps = bc_pool.tile([P, SQ], f32, tag=bc_tag)
        for n in range(2):
            nc.tensor.matmul(
                mb_ps[:, n * 512 : (n + 1) * 512], lhsT=ones_row[:],
                rhs=mean[:, n * 512 : (n + 1) * 512], start=True, stop=True,
            )
            nc.tensor.matmul(
                rb_ps[:, n * 512 : (n + 1) * 512], lhsT=ones_row[:],
                rhs=rstd[:, n * 512 : (n + 1) * 512], start=True, stop=True,
            )
        for j in range(HC):
            t1 = pool.tile([P, SQ], f32, tag="lnt1", bufs=2)
            nc.vector.tensor_tensor(t1[:], src_s[:, j, :], mb_ps[:], OP.subtract)
            t2 = pool.tile([P, SQ], f32, tag="lnt2", bufs=2)
            nc.vector.scalar_tensor_tensor(
                out=t2[:], in0=t1[:], scalar=w_s[:, j : j + 1], in1=rb_ps[:],
                op0=OP.mult, op1=OP.mult,
            )
            nc.vector.tensor_scalar(
                dst_s[:, j, :], t2[:], b_s[:, j : j + 1], None, OP.add,
            )
            if plane_cb is not None:
                plane_cb(j)

    return stats, finish


def _emit(nc, tc, t, mybir, make_identity):
    """Emit the per-core program. `t` maps tensor name -> DRAM AP."""
    from contextlib import ExitStack

    f32 = mybir.dt.float32
    f32r = mybir.dt.float32r
    bf16 = mybir.dt.bfloat16
    AF = mybir.ActivationFunctionType
    OP = mybir.AluOpType

    def mm(ps, lhsT, rhs, start, stop):
        nc.tensor.matmul(ps, lhsT=lhsT, rhs=rhs, start=start, stop=stop)

    with ExitStack() as ctx:
        aux = ctx.enter_context(tc.tile_pool(name="aux", bufs=1))

        def aux_load(name, shape, dtype=f32):
            tl = aux.tile(shape, dtype, tag=name)
            nc.gpsimd.dma_start(tl[:], t[name])
            return tl

        bq_s = aux_load("bq2", [P, HC])
        bk_s = aux_load("bk2", [P, HC])
        bo_s = aux_load("bo2", [P, HC])
        b2_s = aux_load("b22", [P, HC])
        l1w_s = aux_load("l1w", [P, HC])
        l1b_s = aux_load("l1b", [P, HC])
        l2w_s = aux_load("l2w", [P, HC])
        l2b_s = aux_load("l2b", [P, HC])
        b1_s = aux_load("b12", [P, FC])
        bvb_s = aux.tile([P, H], f32)
        nc.gpsimd.dma_start(bvb_s[:], t["bv"].partition_broadcast(P))
        ones_f = aux.tile([P, 1], f32)
        nc.vector.memset(ones_f[:], 1.0)
        ones_s = aux.tile([P, 1], f32r)
        nc.vector.tensor_copy(ones_s[:], ones_f[:])
        ones_rf = aux.tile([1, P], f32)
        nc.vector.memset(ones_rf[:], 1.0)
        ones_row = aux.tile([1, P], f32r)
        nc.vector.tensor_copy(ones_row[:], ones_rf[:])
        zero_s = aux.tile([P, 1], f32)
        nc.vector.memset(zero_s[:], 0.0)
        eps_s = aux.tile([1, 1], f32)
        nc.vector.memset(eps_s[:], EPS)
        ident_s = aux.tile([P, P], f32)
        make_identity(nc, ident_s[:])

        # x1 (LN1 output) outlives the attention/O-proj scopes below.
        keep = ctx.enter_context(tc.tile_pool(name="keep", bufs=1))
        x1_s = keep.tile([P, HC, SQ], f32r)

        with tc.tile_pool(name="resid", bufs=1) as resid:
            xTq_s = resid.tile([P, HC, SQ], f32r)
            for j in range(HC):
                eng = nc.sync if j % 2 == 0 else nc.scalar
                eng.dma_start(
                    xTq_s[:, j, :], t["xTq"][j * P : (j + 1) * P, :]
                )
            with tc.tile_pool(name="attn_out", bufs=1) as aop:
                attnT_s = aop.tile([P, HC, SQ], bf16)

                with tc.tile_pool(name="qkv_keep", bufs=1) as p2:
                    # qTz[p, h, q]: head h's 64 q-rows live at partitions
                    # (h%2)*64..+64 of plane h; the other 64 partitions stay
                    # zero so scores can contract over all 128 partitions.
                    qTz_s = p2.tile([P, NH, SQ], bf16)
                    # only the half-planes Q-proj does not write need zeroing
                    qTz_v = qTz_s[:].rearrange("p (a b) q -> p a b q", b=2)
                    nc.vector.memset(qTz_v[DH:P, :, 0, :], 0.0)
                    nc.vector.memset(qTz_v[0:DH, :, 1, :], 0.0)
                    kT_s = p2.tile([P, HC, S], bf16)
                    # v_s pair layout (stride VW=162):
                    #   [A.dims 0:64 | A.ones 64 | pad | B.ones 66 |
                    #    pad 67:98 | B.dims 98:162]
                    # Stationary window for A = cols base..base+128 (out
                    # rows 0:64 = A attn, row 64 = A softmax sums); for
                    # B = cols base+34..base+162 (out row 32 = B sums,
                    # rows 64:128 = B attn).  Partition-offset rules need
                    # the sums rows 32-aligned (64 for A, 32 for B).
                    v_s = p2.tile([P, TS, NP_ * VW], bf16)
                    v_view = v_s[:].rearrange("p t (c w) -> p t c w", w=VW)
                    nc.vector.memset(v_view[:, :, :, DH : DH + 3], 1.0)

                    # ---------------- QKV projections ----------------
                    with tc.tile_pool(name="qkvph", bufs=1) as ph, tc.tile_pool(
                        name="wstream", bufs=3
                    ) as ws, tc.tile_pool(
                        name="qkv_ps", bufs=3, space="PSUM"
                    ) as pp:
                        xT_s = ph.tile([P, HC, S], bf16)
                        xT_r = t["xT"].rearrange("(c p) s -> p c s", p=P)
                        for j in range(HC):
                            eng = nc.scalar if j % 2 == 0 else nc.sync
                            eng.dma_start(xT_s[:, j, :], xT_r[:, j, :])

                        # Q (our 1024 query tokens; fp32r from the residual copy)
                        for j in range(HC):
                            w_t = ws.tile([P, HC, P], f32r, tag="w")
                            nc.gpsimd.dma_start(
                                w_t[:],
                                t["Wq"][:, j * P : (j + 1) * P].rearrange(
                                    "(c p) m -> p c m", p=P
                                ),
                            )
                            ps = pp.tile([P, SQ], f32, tag="qkps")
                            for kc in range(HC):
                                for n in range(2):
                                    mm(
                                        ps[:, n * 512 : (n + 1) * 512],
                                        w_t[:, kc, :],
                                        xTq_s[:, kc, n * 512 : (n + 1) * 512],
                                        kc == 0,
                                        kc == HC - 1,
                                    )
                            nc.scalar.activation(
                                qTz_s[0:DH, 2 * j, :], ps[0:DH, :],
                                AF.Identity, bias=bq_s[0:DH, j : j + 1],
                            )
                            nc.scalar.activation(
                                qTz_s[DH:P, 2 * j + 1, :], ps[DH:P, :],
                                AF.Identity, bias=bq_s[DH:P, j : j + 1],
                            )

                        # K (all 2048 tokens, bf16)
                        for j in range(HC):
                            wk_t = ws.tile([P, HC, P], bf16, tag="w")
                            nc.gpsimd.dma_start(
                                wk_t[:],
                                t["Wk"][:, j * P : (j + 1) * P].rearrange(
                                    "(c p) m -> p c m", p=P
                                ),
                            )
                            for hf in range(2):
                                ps = pp.tile([P, SQ], f32, tag="qkps")
                                for kc in range(HC):
                                    for n in range(2):
                                        mm(
                                            ps[:, n * 512 : (n + 1) * 512],
                                            wk_t[:, kc, :],
                                            xT_s[
                                                :, kc,
                                                hf * SQ + n * 512 :
                                                hf * SQ + (n + 1) * 512,
                                            ],
                                            kc == 0,
                                            kc == HC - 1,
                                        )
                                nc.scalar.activation(
                                    kT_s[:, j, hf * SQ : (hf + 1) * SQ],
                                    ps[:],
                                    AF.Identity,
                                    bias=bk_s[:, j : j + 1],
                                )

                        # V (token-major, pair layout with ones columns)
                        wv_t = ws.tile([P, HC, H], bf16, tag="wv", bufs=1)
                        nc.gpsimd.dma_start(
                            wv_t[:], t["Wv"].rearrange("(c p) m -> p c m", p=P)
                        )
                        bvb_v = bvb_s[:].rearrange("p (c d) -> p c d", d=2 * DH)
                        for tt in range(TS):
                            ps = pp.tile([P, SQ], f32, tag="qkps")
                            for kc in range(HC):
                                mm(
                                    ps[:, 0:512],
                                    xT_s[:, kc, tt * P : (tt + 1) * P],
                                    wv_t[:, kc, 0:512],
                                    kc == 0,
                                    kc == HC - 1,
                                )
                                mm(
                                    ps[:, 512:H],
                                    xT_s[:, kc, tt * P : (tt + 1) * P],
                                    wv_t[:, kc, 512:H],
                                    kc == 0,
                                    kc == HC - 1,
                                )
                            ps_v = ps[:, 0:H].rearrange(
                                "p (c d) -> p c d", d=2 * DH
                            )
                            nc.vector.scalar_tensor_tensor(
                                out=v_view[:, tt, :, 0:DH],
                                in0=ps_v[:, :, 0:DH],
                                scalar=1.0,
                                in1=bvb_v[:, :, 0:DH],
                                op0=OP.mult,
                                op1=OP.add,
                            )
                            nc.vector.scalar_tensor_tensor(
                                out=v_view[:, tt, :, 98 : 98 + DH],
                                in0=ps_v[:, :, DH : 2 * DH],
                                scalar=1.0,
                                in1=bvb_v[:, :, DH : 2 * DH],
                                op0=OP.mult,
                                op1=OP.add,
                            )

                    # ---------------- attention ----------------
                    # Hybrid exp: the first 5 key-chunks of each head are
                    # exp'd directly PSUM->SBUF by Act; the remaining 11 are
                    # copied PSUM->SBUF bf16 by Vector and mega-exp'd in
                    # place by Act.  This balances the two engines that can
                    # read PSUM (~19us/head each).  AV for head h-1
                    # interleaves block-wise with scores of head h so exp
                    # latency never stalls the PE.
                    BLOCKS = [(0, 7, "direct"), (7, 10, "mega"),
                              (10, 13, "mega"), (13, 16, "mega")]
                    with tc.tile_pool(name="attn_sb", bufs=1) as ab, tc.tile_pool(
                        name="stage", bufs=3
                    ) as stp, tc.tile_pool(
                        name="prp", bufs=10
                    ) as prp, tc.tile_pool(
                        name="sc_ps", bufs=2, space="PSUM"
                    ) as pps, tc.tile_pool(
                        name="av_ps", bufs=2, space="PSUM"
                    ) as ppa:
                        stages = {}
                        avs = {}

                        def emit_scores_b(h, bi):
                            k0, k1, kind = BLOCKS[bi]
                            hc = h // 2
                            tiles = []
                            st = None
                            if kind == "mega":
                                st = stp.tile([P, 4, SQ], bf16, tag="st")
                            for kt in range(k0, k1):
                                sc = pps.tile([P, SQ], f32, tag="sc")
                                lhsT_k = kT_s[:, hc, kt * P : (kt + 1) * P]
                                for n in range(2):
                                    mm(
                                        sc[:, n * 512 : (n + 1) * 512],
                                        lhsT_k,
                                        qTz_s[:, h, n * 512 : (n + 1) * 512],
                                        True,
                                        True,
                                    )
                                if kind == "direct":
                                    pr = prp.tile([P, SQ], bf16, tag="pr")
                                    nc.scalar.activation(
                                        pr[:], sc[:], AF.Exp, bias=0.0,
                                        scale=0.125,
                                    )
                                    tiles.append(pr)
                                else:
                                    nc.vector.tensor_copy(
                                        st[:, kt - k0, :], sc[:]
                                    )
                            if kind == "mega":
                                nc.scalar.activation(
                                    st[:, 0 : k1 - k0, :], st[:, 0 : k1 - k0, :],
                                    AF.Exp, bias=0.0, scale=0.125,
                                )
                            stages[(h, bi)] = (kind, tiles, st)

                        def emit_av_b(h, bi):
                            k0, k1, _ = BLOCKS[bi]
                            hc, par = h // 2, h % 2
                            if bi == 0:
                                av = ppa.tile([P, SQ], f32, tag="av")
                                avs[h] = av
                            else:
                                av = avs[h]
                            kind, tiles, st = stages.pop((h, bi))
                            base = hc * VW + 34 * par
                            for kt in range(k0, k1):
                                pr = tiles[kt - k0] if kind == "direct" else st
                                prs = (
                                    pr[:, :] if kind == "direct"
                                    else pr[:, kt - k0, :]
                                )
                                for n in range(2):
                                    mm(
                                        av[:, n * 512 : (n + 1) * 512],
                                        v_s[:, kt, base : base + P],
                                        prs[:, n * 512 : (n + 1) * 512],
                                        kt == 0,
                                        kt == TS - 1,
                                    )

                        norm_st = {}

                        def emit_norm_a(h):
                            """Spill + sums-row copy + reciprocal chain on
                            the DVE, right after AV(h) completes."""
                            par = h % 2
                            av = avs.pop(h)
                            sp = ab.tile([P, SQ], bf16, tag="sp", bufs=1)
                            nc.vector.tensor_copy(sp[:], av[:])
                            sumrow = DH if par == 0 else 32
                            sv = ab.tile([1, SQ], f32, tag="sums", bufs=1)
                            nc.vector.tensor_copy(
                                sv[:], av[sumrow : sumrow + 1, :]
                            )
                            rec = ab.tile([1, SQ], f32, tag="rec", bufs=1)
                            nc.vector.reciprocal_approx_fast(rec[:], sv[:])
                            rec_r = ab.tile([1, SQ], f32r, tag="recr", bufs=1)
                            nc.vector.tensor_copy(rec_r[:], rec[:])
                            norm_st[h] = (sp, rec_r)

                        def emit_norm_b(h):
                            """Partition-broadcast of the reciprocal row and
                            the normalize multiply.  Emitted a block into the
                            NEXT head so the bc matmul never blocks the PE
                            queue while the reciprocal chain runs."""
                            hc, par = h // 2, h % 2
                            sp, rec_r = norm_st.pop(h)
                            bc = ppa.tile([P, SQ], f32, tag="av")
                            for n in range(2):
                                mm(
                                    bc[:, n * 512 : (n + 1) * 512],
                                    ones_row[:],
                                    rec_r[:, n * 512 : (n + 1) * 512],
                                    True,
                                    True,
                                )
                            rows = slice(0, DH) if par == 0 else slice(DH, P)
                            nc.vector.tensor_tensor(
                                attnT_s[rows, hc, :], sp[rows, :], bc[rows, :],
                                OP.mult,
                            )

                        for h in range(NH):
                            for bi in range(len(BLOCKS)):
                                emit_scores_b(h, bi)
                                if bi == 1 and h > 1:
                                    emit_norm_b(h - 2)
                                if h > 0:
                                    emit_av_b(h - 1, bi)
                            if h > 0:
                                emit_norm_a(h - 1)
                        for bi in range(len(BLOCKS)):
                            emit_av_b(NH - 1, bi)
                            if bi == 1:
                                emit_norm_b(NH - 2)
                        emit_norm_a(NH - 1)
                        emit_norm_b(NH - 1)

                # ---------------- O-projection + residual + LN1 ----------------
                with tc.tile_pool(name="oproj", bufs=1) as op_, tc.tile_pool(
                    name="wo_st", bufs=3
                ) as wop, tc.tile_pool(
                    name="o_ps", bufs=2, space="PSUM"
                ) as ppo, tc.tile_pool(
                    name="st_ps", bufs=1, space="PSUM"
                ) as ppst:
                    r1_s = op_.tile([P, HC, SQ], f32r)
                    ln1_stats, ln1_finish = _ln_make(
                        nc, mybir, op_, ppst, ppo, ones_row, eps_s,
                        r1_s, x1_s, l1w_s, l1b_s, ones_s,
                    )
                    for j in range(HC):
                        wo_t = wop.tile([P, HC, P], bf16, tag="wo")
                        nc.gpsimd.dma_start(
                            wo_t[:],
                            t["Wo"][:, j * P : (j + 1) * P].rearrange(
                                "(c p) m -> p c m", p=P
                            ),
                        )
                        ps = ppo.tile([P, SQ], f32, tag="ops")
                        for kc in range(HC):
                            for n in range(2):
                                mm(
                                    ps[:, n * 512 : (n + 1) * 512],
                                    wo_t[:, kc, :],
                                    attnT_s[
                                        :, kc, n * 512 : (n + 1) * 512
                                    ],
                                    kc == 0,
                                    kc == HC - 1,
                                )
                        nc.vector.scalar_tensor_tensor(
                            out=r1_s[:, j, :],
                            in0=ps[:],
                            scalar=bo_s[:, j : j + 1],
                            in1=xTq_s[:, j, :],
                            op0=OP.add,
                            op1=OP.add,
                        )
                        ln1_stats(j)
                    ln1_finish()

        # ---------------- MLP + LN2 + output ----------------
        # W2 runs kc-major in three j-pair passes; the first pass
        # accumulates each hT plane right behind its gelu, so W2's first
        # third is hidden under W1 and there is no W1->W2 barrier.
        with tc.tile_pool(name="mlp", bufs=1) as mp, tc.tile_pool(
            name="w1_st", bufs=3
        ) as w1p, tc.tile_pool(
            name="acc_ps", bufs=2, space="PSUM"
        ) as accp:
            hT_s = mp.tile([P, FC, SQ], bf16)
            r2_s = mp.tile([P, HC, SQ], f32r)
            w2_s = mp.tile([P, FC, H], bf16)
            w2_r = t["W2"].rearrange("(c p) m -> p c m", p=P)
            for kc in range(FC):
                eng = nc.scalar if kc % 2 == 0 else nc.sync
                eng.dma_start(w2_s[:, kc, :], w2_r[:, kc, :])

            def w2_terms(accs, jpair, m):
                for jj in range(2):
                    j = 2 * jpair + jj
                    for n in range(2):
                        mm(
                            accs[jj][:, n * 512 : (n + 1) * 512],
                            w2_s[:, m, j * P : (j + 1) * P],
                            hT_s[:, m, n * 512 : (n + 1) * 512],
                            m == 0,
                            m == FC - 1,
                        )

            def w2_emit_r2(accs, jpair):
                for jj in range(2):
                    j = 2 * jpair + jj
                    nc.vector.scalar_tensor_tensor(
                        out=r2_s[:, j, :],
                        in0=accs[jj][:],
                        scalar=b2_s[:, j : j + 1],
                        in1=x1_s[:, j, :],
                        op0=OP.add,
                        op1=OP.add,
                    )

            with tc.tile_pool(name="m_ps", bufs=2, space="PSUM") as ppm:
                acc_a = accp.tile([P, SQ], f32, tag="acc")
                acc_b = accp.tile([P, SQ], f32, tag="acc")
                accs0 = [acc_a, acc_b]
                for m in range(FC):
                    w1_t = w1p.tile([P, HC, P], f32r, tag="w1")
                    nc.gpsimd.dma_start(
                        w1_t[:],
                        t["W1"][:, m * P : (m + 1) * P].rearrange(
                            "(c p) n -> p c n", p=P
                        ),
                    )
                    ps = ppm.tile([P, SQ], f32, tag="mps")
                    for kc in range(HC):
                        for n in range(2):
                            mm(
                                ps[:, n * 512 : (n + 1) * 512],
                                w1_t[:, kc, :],
                                x1_s[:, kc, n * 512 : (n + 1) * 512],
                                kc == 0,
                                kc == HC - 1,
                            )
                    nc.scalar.activation(
                        hT_s[:, m, :], ps[:], AF.Gelu, bias=b1_s[:, m : m + 1]
                    )
                    if m > 0:
                        w2_terms(accs0, 0, m - 1)
                w2_terms(accs0, 0, FC - 1)
                w2_emit_r2(accs0, 0)

            with tc.tile_pool(name="st2_ps", bufs=1, space="PSUM") as ppst2:
                ln2_stats, ln2_finish = _ln_make(
                    nc, mybir, mp, ppst2, accp, ones_row, eps_s,
                    r2_s, r2_s, l2w_s, l2b_s, ones_s, bc_tag="acc",
                )
                ln2_stats(0)
                ln2_stats(1)
                for jpair in (1, 2):
                    acc_a = accp.tile([P, SQ], f32, tag="acc")
                    acc_b = accp.tile([P, SQ], f32, tag="acc")
                    accs = [acc_a, acc_b]
                    for m in range(FC):
                        w2_terms(accs, jpair, m)
                    w2_emit_r2(accs, jpair)
                    ln2_stats(2 * jpair)
                    ln2_stats(2 * jpair + 1)
                ln2_finish()

            # transpose back to token-major and store
            with tc.tile_pool(name="outp", bufs=2) as outp, tc.tile_pool(
                name="tr_ps", bufs=4, space="PSUM"
            ) as ppt:
                for tt in range(TQ):
                    out_t = outp.tile([P, H], f32, tag="out")
                    for j in range(HC):
                        tps = ppt.tile([P, P], f32, tag="tr")
                        nc.tensor.transpose(
                            tps[:],
                            r2_s[:, j, tt * P : (tt + 1) * P].bitcast(f32),
                            ident_s[:],
                        )
                        if j % 2 == 0:
                            nc.scalar.activation(
                                out_t[:, j * P : (j + 1) * P], tps[:],
                                AF.Identity, bias=zero_s[:],
                            )
                        else:
                            nc.vector.tensor_copy(
                                out_t[:, j * P : (j + 1) * P], tps[:]
                            )
                    nc.sync.dma_start(
                        t["y"][tt * P : (tt + 1) * P, :], out_t[:]
                    )


def _build():
    import concourse.bacc as bacc
    import concourse.tile as tile
    import concourse.mybir as mybir
    from concourse.masks import make_identity

    f32 = mybir.dt.float32
    f32r = mybir.dt.float32r
    bf16 = mybir.dt.bfloat16

    nc = bacc.Bacc(
        "TRN2", target_bir_lowering=False, debug=False, num_devices=N_CORES
    )
    specs = [
        ("xT", [H, S], bf16, "ExternalInput"),
        ("xTq", [H, SQ], f32r, "ExternalInput"),
        ("Wq", [H, H], f32r, "ExternalInput"),
        ("Wk", [H, H], bf16, "ExternalInput"),
        ("Wv", [H, H], bf16, "ExternalInput"),
        ("Wo", [H, H], bf16, "ExternalInput"),
        ("W1", [H, FF], f32r, "ExternalInput"),
        ("W2", [FF, H], bf16, "ExternalInput"),
        ("bq2", [P, HC], f32, "ExternalInput"),
        ("bk2", [P, HC], f32, "ExternalInput"),
        ("bv", [H], f32, "ExternalInput"),
        ("bo2", [P, HC], f32, "ExternalInput"),
        ("b12", [P, FC], f32, "ExternalInput"),
        ("b22", [P, HC], f32, "ExternalInput"),
        ("l1w", [P, HC], f32, "ExternalInput"),
        ("l1b", [P, HC], f32, "ExternalInput"),
        ("l2w", [P, HC], f32, "ExternalInput"),
        ("l2b", [P, HC], f32, "ExternalInput"),
        ("y", [SQ, H], f32, "ExternalOutput"),
    ]
    t = {
        name: nc.dram_tensor(name, shape, dt, kind=kind).ap()
        for name, shape, dt, kind in specs
    }
    with tile.TileContext(nc) as tc:
        _emit(nc, tc, t, mybir, make_identity)
    nc.compile()
    return nc


def _chunk_major(v):
    """[C*P] -> [P, C] with entry [p, c] = v[c*P + p]."""
    return np.ascontiguousarray(v.reshape(-1, P).T)


def prepare_in_maps(inputs):
    inp = {k: np.asarray(v) for k, v in inputs.items()}
    x = inp["x"].astype(np.float32)

    shared = {
        "Wq": inp["Wq"].astype(np.float32),
        "Wk": inp["Wk"].astype(BF16),
        "Wv": inp["Wv"].astype(BF16),
        "Wo": inp["Wo"].astype(BF16),
        "W1": inp["W1"].astype(np.float32),
        "W2": inp["W2"].astype(BF16),
        "bq2": _chunk_major(inp["bq"].astype(np.float32)),
        "bk2": _chunk_major(inp["bk"].astype(np.float32)),
        "bv": inp["bv"].astype(np.float32),
        "bo2": _chunk_major(inp["bo"].astype(np.float32)),
        "b12": _chunk_major(inp["b1"].astype(np.float32)),
        "b22": _chunk_major(inp["b2"].astype(np.float32)),
        "l1w": _chunk_major(inp["ln1_w"].astype(np.float32)),
        "l1b": _chunk_major(inp["ln1_b"].astype(np.float32)),
        "l2w": _chunk_major(inp["ln2_w"].astype(np.float32)),
        "l2b": _chunk_major(inp["ln2_b"].astype(np.float32)),
    }
    in_maps = []
    for c in range(N_CORES):
        b, hf = c // 2, c % 2
        xT = np.ascontiguousarray(x[b].T)
        m = dict(shared)
        m["xT"] = xT.astype(BF16)
        m["xTq"] = np.ascontiguousarray(xT[:, hf * SQ : (hf + 1) * SQ])
        in_maps.append(m)
    return in_maps


def get_program():
    if "nc" not in _CACHE:
        _CACHE["nc"] = _build()
    return _CACHE["nc"]


def kernel(**inputs):
    from concourse.bass_utils import run_bass_kernel_spmd

    nc = get_program()
    in_maps = prepare_in_maps(inputs)
    res = run_bass_kernel_spmd(nc, in_maps, core_ids=list(range(N_CORES)))
    out = np.empty((B, S, H), np.float32)
    for c in range(N_CORES):
        b, hf = c // 2, c % 2
        out[b, hf * SQ : (hf + 1) * SQ] = res.results[c]["y"]
    return out
